# revision 41
# baseline (speedup 1.0000x reference)
"""Causal attention kernel for TRN2, 8 NeuronCores (SPMD).

Problem:  x[4096,2048] f32; q = x@Wq.T, k = x@Wk.T (d_head=128),
          scores = q@k.T causal-masked, attn = softmax(scores),
          out = (attn @ x) @ W2.T.

Sharding: sequence-parallel over queries with stride-8 interleave:
  core c owns queries {8m+c : m=0..511}.  For key tile kt (128 keys),
  every core has exactly 512-16*kt valid queries -- a contiguous tail
  slice of its query columns -- so the SPMD program is identical on all
  cores (no dynamic control flow, no collectives) and causal work is
  perfectly balanced.

Precision: fp16 inputs for the q/k projections and the score matmul
  (fp32 PSUM accumulation; scores staged through SBUF in fp16, |s|<~70
  so fp16 quantization adds ~1e-3 rel err), unnormalized softmax (exp
  without max-subtraction), attention weights in bf16 (fp32 exponent
  range, needed for exp(s) up to ~1e28), V and W2 matmuls in bf16/fp16,
  normalization by the softmax row-sum at the attn_out eviction.

Scheduling notes (final, ~199.4us on HW; baseline 212.5us; rel err
3.6e-3 vs 2e-2 gate):
  * Queues: sync carries ONLY the critical in-order byte stream
    [wk, xtp 0..15, xv 0..31, outT]; scalar carries [wq, xq, mask,
    w2r 0..15] in parallel.  gpsimd (SWDGE) measured ~half-speed per
    transfer -- don't use it for bulk.  A single HWDGE queue fans out
    over all 16 DMA engines and sustains ~410 GB/s when issues aren't
    consumer-gated.
  * xtp in KG=256 groups (1MB per DMA): the stream is consumer-gated,
    so per-DMA roundtrip amortizes over 2x bytes; ALSO halves the psk
    PSUM per-buf footprint.
  * The fused loop is software-pipelined for the in-order PE:
    scores(kg) emitted after kproj(kg+2), denominator/fused-V for kg
    after kproj(kg+3).  Scores are copied PSUM->SBUF (fp16)
    immediately, so the single pss bank frees after one vector hop and
    mask/exp run off the SBUF copy, off the PE critical path.
  * PSUM (bank-granular!): fused phase = psk 2 + pss 1 + psd 1 +
    fused-V[0:4) 4 = 8.  V restructure: bcl (oc8-11, FULL kt range)
    takes the 4 left banks the fused pools release and runs
    kt-synchronous with the xv stream; g2 (oc4-7, kt<16, resident xv)
    interleaved with bcl's second half on the right banks freed by the
    fused-V evictions; bcr (oc12-15, full range) after g2; g4 (oc0-7,
    kt>=16) + merges after bcl closes.  bcl evictions are emitted
    BEFORE g2/bcr evictions (vector queue is in-order) so ao8-11 are
    ready when W2 starts.
  * All 16 w2r loads pre-issued on scalar, first gated behind the last
    xv DMA (bytes land in the stream gap); W2's first four oc-groups
    run their ao8-15 half-groups up-front across all 4 ps4 banks to
    hide the g4-merge tail.
  * The denominator PSUM row is copied to SBUF before the (slow,
    1-partition, ~3us) reciprocal so the psd bank frees for bcl.
  * Rejected variants (measured): AllGather-sharded kT (~100us),
    two-queue xtp split via gpsimd (SWDGE slow), xv interleaved into
    the consumer-gated xtp tail (stretches both streams), PE clock
    warm-up via dummy matmuls (runs at cold clock, delays real work),
    fp8 anywhere (e4m3 ~2.7% per matmul vs 2% gate).
"""

from contextlib import ExitStack

import numpy as np
import ml_dtypes

import concourse.bass as bass
import concourse.bacc as bacc
import concourse.mybir as mybir
import concourse.tile as tile
from concourse.bass_utils import run_bass_kernel_spmd
from concourse.tile_rust import add_dep_helper

N_CTX = 4096
D_MODEL = 2048
D_HEAD = 128
NCORES = 8
QPC = N_CTX // NCORES          # 512 queries per core
NKT = N_CTX // 128             # 32 key tiles
NDM = D_MODEL // 128           # 16 d_model chunks
KG = 256                       # kT projection key-group width
NKG = N_CTX // KG
MASK_NEG = -30000.0

F16 = mybir.dt.float16
BF16 = mybir.dt.bfloat16
F32 = mybir.dt.float32


def _widths():
    # valid query-column width per key tile (tail slice [512-w : 512] of qT)
    return [QPC - 16 * kt for kt in range(NKT)]


def build_program():
    nc = bacc.Bacc(trn_type="TRN2", target_bir_lowering=False, debug=False)

    # ---- DRAM parameters (identical shapes on all cores; data differs) ----
    # xqr[r, 512*ic + m] = x[8m+c, 128*ic + r]   (own-query columns, packed)
    xqr = nc.declare_dram_parameter("xqr", [128, NDM * QPC], F16, isOutput=False)
    # xtp[kg][r, KG*ic + n] = x[KG*kg + n, 128*ic + r]  (contiguous per-kg tiles)
    xtp = nc.declare_dram_parameter("xtp", [NKG, 128, NDM * KG], F16, isOutput=False)
    # xv = x (natural layout), bf16
    xv = nc.declare_dram_parameter("xv", [N_CTX, D_MODEL], BF16, isOutput=False)
    # wqr[r, 128*ic + h] = Wq[h, 128*ic + r]; same for wkr
    wqr = nc.declare_dram_parameter("wqr", [128, D_MODEL], F16, isOutput=False)
    wkr = nc.declare_dram_parameter("wkr", [128, D_MODEL], F16, isOutput=False)
    # w2r[oc][r, 128*ic + o] = W2[128*oc + o, 128*ic + r]
    w2r = nc.declare_dram_parameter("w2r", [NDM, 128, D_MODEL], F16, isOutput=False)
    maskb = nc.declare_dram_parameter("maskb", [128, 16], F32, isOutput=False)
    outT = nc.declare_dram_parameter("outT", [D_MODEL, QPC], F16, isOutput=True)

    W = _widths()

    with tile.TileContext(nc) as tc:
        with (
            tc.tile_pool(name="static", bufs=1) as st,
            tc.tile_pool(name="xvpool", bufs=NKT) as xvp,
            tc.tile_pool(name="ktpool", bufs=4) as ktp,
            tc.tile_pool(name="atpool", bufs=1) as atp,
        ):
            qT_sb = st.tile([128, QPC], F16, tag="qT")
            ones_sb = st.tile([128, 1], BF16, tag="ones")
            mask_sb = st.tile([128, 16], F32, tag="mask")
            recip_sb = st.tile([128, QPC], F32, tag="recip")
            nc.vector.memset(ones_sb[:], 1.0)

            # ---- byte plan: sync carries ONLY the critical in-order stream
            # [wk, xtp 0..15, xv 0..31, outs]; the q-proj inputs (mask, wq,
            # xq) ride the scalar queue in parallel during the cold start. ----
            es1 = ExitStack()  # SBUF transients: p1 + xts (freed before p34)
            p1 = es1.enter_context(tc.tile_pool(name="p1", bufs=1))
            # wk in two halves: k-proj's first matmuls gate on ~0.4MB instead
            # of the full wk+xtp[0] 1.5MB -- the first ~10us of DMA crawl at
            # ~30 GB/s makes every early byte precious.
            wk_sb = p1.tile([128, D_MODEL], F16, tag="wk")
            nc.sync.dma_start(out=wk_sb[:, : D_MODEL // 2], in_=wkr[:, : D_MODEL // 2])
            wq_sb = p1.tile([128, D_MODEL], F16, tag="wq")
            nc.scalar.dma_start(out=wq_sb[:], in_=wqr[:])
            xq_sb = p1.tile([128, NDM * QPC], F16, tag="xq")
            for qq in range(2):
                nc.scalar.dma_start(
                    out=xq_sb[:, 8 * QPC * qq : 8 * QPC * (qq + 1)],
                    in_=xqr[:, 8 * QPC * qq : 8 * QPC * (qq + 1)],
                )
            nc.scalar.dma_start(out=mask_sb[:], in_=maskb[:])

            # xtp granularity is KG=256 keys (1MB per DMA): the stream is
            # consumer-gated, so per-DMA roundtrip (sem+issue+DGE+sem ~3us)
            # amortizes over 2x the bytes vs 128-key tiles -> ~350-400 GB/s.
            xts = es1.enter_context(tc.tile_pool(name="xts", bufs=4))
            xts_t = [None] * NKG
            # xtp[0] in four quarter-chunks interleaved with wk's second
            # half: kproj(0) steps through its ic-slices as they land.
            t0 = xts.tile([128, NDM * KG], F16, tag="xts", name="xts0")
            Q4 = NDM * KG // 4
            nc.sync.dma_start(out=t0[:, :Q4], in_=xtp[0][:, :Q4])
            nc.sync.dma_start(out=t0[:, Q4 : 2 * Q4], in_=xtp[0][:, Q4 : 2 * Q4])
            nc.sync.dma_start(
                out=wk_sb[:, D_MODEL // 2 :], in_=wkr[:, D_MODEL // 2 :]
            )
            nc.sync.dma_start(out=t0[:, 2 * Q4 : 3 * Q4], in_=xtp[0][:, 2 * Q4 : 3 * Q4])
            nc.sync.dma_start(out=t0[:, 3 * Q4 :], in_=xtp[0][:, 3 * Q4 :])
            xts_t[0] = t0
            for kg in range(1, NKG):
                t = xts.tile([128, NDM * KG], F16, tag="xts", name=f"xts{kg}")
                nc.sync.dma_start(out=t[:], in_=xtp[kg])
                xts_t[kg] = t

            # ---- xv loads strictly AFTER the xT stream (same sync queue, so
            # queue order == byte order; interleaving xv into the consumer-
            # gated xtp tail measured ~7us slower -- it stretches both
            # streams) ----
            xv_t = [None] * NKT
            last_xv_dma = None
            for kt in range(NKT):
                t = xvp.tile([128, D_MODEL], BF16, tag="xv", name=f"xv{kt}")
                last_xv_dma = nc.sync.dma_start(
                    out=t[:], in_=xv[128 * kt : 128 * (kt + 1), :]
                )
                xv_t[kt] = t

            # ---- k-proj for kg 0,1 hoisted BEFORE q-proj: the PE runs its
            # program in order, and xtp[0:2] bytes land before xq, so this
            # keeps the PE busy ~5us earlier than qproj-first. ----
            ktile_t = [None] * NKG
            with tc.tile_pool(name="psk", bufs=2, space="PSUM") as pskp:

                def emit_kproj(kg):
                    psk = pskp.tile([128, KG], F32, tag="psk", name=f"psk{kg}")
                    for ic in range(NDM):
                        nc.tensor.matmul(
                            psk[:],
                            wk_sb[:, 128 * ic : 128 * (ic + 1)],
                            xts_t[kg][:, KG * ic : KG * (ic + 1)],
                            start=(ic == 0), stop=(ic == NDM - 1),
                        )
                    ktile = ktp.tile([128, KG], F16, tag="kt", name=f"kt{kg}")
                    nc.vector.tensor_copy(ktile[:], psk[:])
                    ktile_t[kg] = ktile

                emit_kproj(0)
                emit_kproj(1)

                # ---- qT projection (2 halves, gated on the 2 xq chunks) ----
                with tc.tile_pool(name="psq", bufs=1, space="PSUM") as psqp:
                    psq = psqp.tile([128, QPC], F32, tag="psq")
                    for ic in range(NDM):
                        nc.tensor.matmul(
                            psq[:],
                            wq_sb[:, 128 * ic : 128 * (ic + 1)],
                            xq_sb[:, QPC * ic : QPC * (ic + 1)],
                            start=(ic == 0), stop=(ic == NDM - 1),
                        )
                    nc.vector.tensor_copy(qT_sb[:], psq[:])

                # ---- fused pipeline: kT proj / scores / exp / denom / V[0:4] ----
                es2 = ExitStack()  # PSUM: psv1 (freed mid-way through V chunk waves)
                psv1 = es2.enter_context(
                    tc.tile_pool(name="psv1", bufs=4, space="PSUM", side="right")
                )
                pso1 = [
                    psv1.tile([128, QPC], F32, tag="pso1", name=f"pso1_{j}")
                    for j in range(4)
                ]
                at_t = []
                scp = es1.enter_context(tc.tile_pool(name="scp", bufs=2))
                with (
                    tc.tile_pool(name="pss", bufs=1, space="PSUM") as pssp,
                    tc.tile_pool(name="psd", bufs=1, space="PSUM") as psdp,
                ):
                    psd = psdp.tile([1, QPC], F32, tag="psd")

                    def emit_scores(kg):
                        # scores -> SBUF copy -> mask -> exp for group kg.
                        # The PSUM score tile is copied to SBUF right away --
                        # the pss bank frees after one vector hop, and
                        # mask/exp run off the SBUF copy.
                        ktile = ktile_t[kg]
                        for sub in range(KG // 128):
                            kt = kg * (KG // 128) + sub
                            w = W[kt]
                            ps = pssp.tile([128, 512], F32, tag="pss", name=f"pss{kt}")
                            nc.tensor.matmul(
                                ps[:, :w],
                                ktile[:, 128 * sub : 128 * (sub + 1)],
                                qT_sb[:, QPC - w : QPC],
                                start=True, stop=True,
                            )
                            sc = scp.tile([128, 512], F16, tag="sc", name=f"sc{kt}")
                            nc.vector.tensor_copy(sc[:, :w], ps[:, :w])
                            nc.vector.tensor_add(sc[:, :16], sc[:, :16], mask_sb[:])
                            at = atp.tile([128, w], BF16, tag=f"at{kt}")
                            nc.scalar.activation(
                                at[:], sc[:, :w], mybir.ActivationFunctionType.Exp
                            )
                            at_t.append(at)

                    def emit_atv(kg):
                        # denominator + fused-V matmuls for group kg -- the
                        # at-tile consumers.  Emitted one group later than
                        # emit_scores so the PE (in-order) never waits on its
                        # own group's exp chain.
                        for sub in range(KG // 128):
                            kt = kg * (KG // 128) + sub
                            w = W[kt]
                            at = at_t[kt]
                            nc.tensor.matmul(
                                psd[0:1, QPC - w : QPC],
                                ones_sb[:],
                                at[:],
                                start=(kt == 0), stop=(kt == NKT - 1),
                            )
                            # V matmul for output chunks 0-3, fused
                            # (first key half only; the early eviction at xv[15]
                            # frees all 8 banks for the oc 8-15 full streams)
                            if kt < NKT // 2:
                                for j in range(4):
                                    nc.tensor.matmul(
                                        pso1[j][:, QPC - w : QPC],
                                        xv_t[kt][:, 128 * j : 128 * (j + 1)],
                                        at[:],
                                        start=(kt == 0), stop=(kt == NKT // 2 - 1),
                                    )

                    # software-pipelined emission: scores(kg) trail
                    # kproj(kg+2) and the at-consumers trail kproj(kg+3), so
                    # both the DMA->kproj and score->exp roundtrips hide
                    # behind kproj work (the PE executes its program in
                    # order).
                    for kg in range(2, NKG):
                        emit_kproj(kg)
                        emit_scores(kg - 2)
                        if kg >= 3:
                            emit_atv(kg - 3)
                    emit_scores(NKG - 2)
                    emit_atv(NKG - 3)
                    emit_scores(NKG - 1)
                    emit_atv(NKG - 2)
                    emit_atv(NKG - 1)

                    # copy the denominator row to SBUF immediately -- the psd
                    # bank is one of the four the bcl streams need, and the
                    # 1-partition reciprocal is slow (~3us); run it off SBUF
                    # (staged in a spare row of recip_sb) after the pool
                    # closes.
                    nc.vector.tensor_copy(recip_sb[0:1, :], psd[0:1, :])

            nc.vector.reciprocal(recip_sb[0:1, :], recip_sb[0:1, :])
            es1.close()  # free p1 + xts SBUF for the aoT / W2 pools
            nc.gpsimd.partition_broadcast(recip_sb[:], recip_sb[0:1, :])

            HK = NKT // 2
            with tc.tile_pool(name="p34", bufs=1) as p34:
                ao_t = {}

                # ---- oc 8-11 FULL-range V streams on the 4 left banks the
                # fused pools released, kt-synchronous with the xv stream
                # (xv tiles 0-15 are interleaved into the xtp tail, so these
                # start before the xT stream finishes) ----
                with tc.tile_pool(name="bcl", bufs=4, space="PSUM") as bclp:
                    bcl = {
                        oc: bclp.tile([128, QPC], F32, tag="bcl", name=f"bcl_{oc}")
                        for oc in range(8, 12)
                    }
                    for kt in range(HK):
                        w = W[kt]
                        for oc in range(8, 12):
                            nc.tensor.matmul(
                                bcl[oc][:, QPC - w : QPC],
                                xv_t[kt][:, 128 * oc : 128 * (oc + 1)],
                                at_t[kt][:],
                                start=(kt == 0), stop=False,
                            )
                    # early normalized evictions for oc 0-3 (first key half;
                    # exact for queries m<256 by causality), freeing the
                    # right-side fused-V banks
                    for j in range(4):
                        t = p34.tile([128, QPC], F16, tag=f"ao{j}")
                        nc.vector.tensor_mul(t[:], pso1[j][:], recip_sb[:])
                        ao_t[j] = t
                    es2.close()  # release the fused V banks

                    # ---- oc 4-7 first-half streams on the freed right banks,
                    # interleaved with bcl's second half: g2 reads resident
                    # xv[0:16] so the PE has work while the xv tail lands ----
                    with tc.tile_pool(name="g2", bufs=4, space="PSUM", side="right") as g2p:
                        g2 = {
                            oc: g2p.tile([128, QPC], F32, tag="g2", name=f"g2_{oc}")
                            for oc in range(4, 8)
                        }
                        for kt in range(HK, NKT):
                            w = W[kt]
                            for oc in range(8, 12):
                                nc.tensor.matmul(
                                    bcl[oc][:, QPC - w : QPC],
                                    xv_t[kt][:, 128 * oc : 128 * (oc + 1)],
                                    at_t[kt][:],
                                    start=False, stop=(kt == NKT - 1),
                                )
                            kg2 = kt - HK
                            wg = W[kg2]
                            for oc in range(4, 8):
                                nc.tensor.matmul(
                                    g2[oc][:, QPC - wg : QPC],
                                    xv_t[kg2][:, 128 * oc : 128 * (oc + 1)],
                                    at_t[kg2][:],
                                    start=(kg2 == 0), stop=(kg2 == HK - 1),
                                )
                        # bcl evictions FIRST (its stop lands with the last xv
                        # tile; the vector queue is in-order, so emitting them
                        # before g2/bcr evicts makes ao8-11 available ~25us
                        # earlier for W2's first groups)
                        for oc in range(8, 12):
                            t = p34.tile([128, QPC], F16, tag=f"ao{oc}")
                            nc.vector.tensor_mul(t[:], bcl[oc][:], recip_sb[:])
                            ao_t[oc] = t
                        for oc in range(4, 8):
                            t = p34.tile([128, QPC], F16, tag=f"ao{oc}")
                            nc.vector.tensor_mul(t[:], g2[oc][:], recip_sb[:])
                            ao_t[oc] = t

                    # ---- oc 12-15 full range on the right banks ----
                    with tc.tile_pool(name="bcr", bufs=4, space="PSUM", side="right") as bcrp:
                        bcr = {
                            oc: bcrp.tile([128, QPC], F32, tag="bcr", name=f"bcr_{oc}")
                            for oc in range(12, NDM)
                        }
                        for kt in range(NKT):
                            w = W[kt]
                            for oc in range(12, NDM):
                                nc.tensor.matmul(
                                    bcr[oc][:, QPC - w : QPC],
                                    xv_t[kt][:, 128 * oc : 128 * (oc + 1)],
                                    at_t[kt][:],
                                    start=(kt == 0), stop=(kt == NKT - 1),
                                )
                        for oc in range(12, NDM):
                            t = p34.tile([128, QPC], F16, tag=f"ao{oc}")
                            nc.vector.tensor_mul(t[:], bcr[oc][:], recip_sb[:])
                            ao_t[oc] = t

                # ---- oc 0-7, second key half (kt 16-31, queries [256:512]
                # only): dense from resident xv, merged into the early aos ----
                with (
                    tc.tile_pool(name="g4", bufs=4, space="PSUM") as g4p,
                    tc.tile_pool(name="tmr", bufs=4) as tmr,
                ):
                    for ocs in (range(0, 4), range(4, 8)):
                        g4 = {
                            oc: g4p.tile(
                                [128, QPC // 2], F32, tag="g4", name=f"g4_{oc}"
                            )
                            for oc in ocs
                        }
                        for kt in range(HK, NKT):
                            w = W[kt]
                            for oc in ocs:
                                nc.tensor.matmul(
                                    g4[oc][:, QPC // 2 - w : QPC // 2],
                                    xv_t[kt][:, 128 * oc : 128 * (oc + 1)],
                                    at_t[kt][:],
                                    start=(kt == HK), stop=(kt == NKT - 1),
                                )
                        for oc in ocs:
                            tm = tmr.tile([128, QPC // 2], F16, tag="tm")
                            nc.vector.tensor_mul(
                                tm[:], g4[oc][:], recip_sb[:, QPC // 2 :]
                            )
                            nc.vector.tensor_add(
                                ao_t[oc][:, QPC // 2 :],
                                ao_t[oc][:, QPC // 2 :],
                                tm[:],
                            )

                    # ---- W2: outT = W2T.T @ attn_outT.  ic order 8..15 first
                    # (those aos finish earliest), 0..7 after the merges ----
                    # All 16 w2 weight-load issues go on the scalar queue
                    # UPFRONT (ordered after the xv stream; tiles 7+ gate on
                    # buffer reuse) so no out-DMA issue can head-of-line
                    # block them; out DMAs ride the vector queue instead.
                    with (
                        tc.tile_pool(name="w2s", bufs=8) as w2s,
                        tc.tile_pool(name="outs", bufs=4) as outs,
                        tc.tile_pool(name="ps4", bufs=4, space="PSUM", side="right") as ps4,
                    ):
                        tw_t = []
                        for oc in range(NDM):
                            tw = w2s.tile([128, D_MODEL], F16, tag="w2",
                                          name=f"w2_{oc}")
                            dw = nc.scalar.dma_start(out=tw[:], in_=w2r[oc])
                            if oc == 0:
                                add_dep_helper(dw.ins, last_xv_dma.ins,
                                               reason="w2 bytes after xv stream")
                            tw_t.append(tw)
                        # The first four ocs run their ao8-15 half-groups
                        # up-front across all four banks (~13us of work whose
                        # inputs are ready at W2 start), hiding the tail of
                        # the g4 merges that ao0-7 still need.
                        ic_order = list(range(8, NDM)) + list(range(0, 8))
                        ps_first = {}
                        for oc in range(4):
                            ps_first[oc] = ps4.tile(
                                [128, QPC], F32, tag="ps4", name=f"ps4_{oc}"
                            )
                            for i, ic in enumerate(ic_order[:8]):
                                nc.tensor.matmul(
                                    ps_first[oc][:],
                                    tw_t[oc][:, 128 * ic : 128 * (ic + 1)],
                                    ao_t[ic][:],
                                    start=(i == 0), stop=False,
                                )
                        for oc in range(NDM):
                            if oc < 4:
                                ps = ps_first[oc]
                                for i, ic in enumerate(ic_order[8:]):
                                    nc.tensor.matmul(
                                        ps[:],
                                        tw_t[oc][:, 128 * ic : 128 * (ic + 1)],
                                        ao_t[ic][:],
                                        start=False, stop=(i == 7),
                                    )
                            else:
                                ps = ps4.tile([128, QPC], F32, tag="ps4")
                                for i, ic in enumerate(ic_order):
                                    nc.tensor.matmul(
                                        ps[:],
                                        tw_t[oc][:, 128 * ic : 128 * (ic + 1)],
                                        ao_t[ic][:],
                                        start=(i == 0), stop=(i == NDM - 1),
                                    )
                            t = outs.tile([128, QPC], F16, tag="out")
                            if oc == NDM - 1:
                                # split the final eviction so the copy/DMA
                                # chain pipelines instead of serializing the
                                # whole tail after the last matmul
                                for h in range(2):
                                    sl = slice(h * QPC // 2, (h + 1) * QPC // 2)
                                    nc.vector.tensor_copy(t[:, sl], ps[:, sl])
                                    nc.sync.dma_start(
                                        out=outT[128 * oc : 128 * (oc + 1), sl],
                                        in_=t[:, sl],
                                    )
                            else:
                                nc.vector.tensor_copy(t[:], ps[:])
                                nc.sync.dma_start(
                                    out=outT[128 * oc : 128 * (oc + 1), :], in_=t[:]
                                )

    nc.compile()
    return nc


def prepare_inputs(x, Wk, Wq, W2):
    """Host-side sharding/layout prep. Returns in_maps for the 8 cores."""
    x = np.asarray(x, dtype=np.float32)
    Wk = np.asarray(Wk, dtype=np.float32)
    Wq = np.asarray(Wq, dtype=np.float32)
    W2 = np.asarray(W2, dtype=np.float32)

    xT16 = np.ascontiguousarray(x.T).astype(np.float16)          # [D, N]
    # xtp[kg, r, KG*ic + n] = xT[128*ic + r, KG*kg + n]
    xtp = np.ascontiguousarray(
        xT16.reshape(NDM, 128, NKG, KG).transpose(2, 1, 0, 3).reshape(NKG, 128, NDM * KG)
    )
    xv16 = x.astype(ml_dtypes.bfloat16)                          # [N, D]

    def pack_chunks(aT, width):
        # aT [D_MODEL, width] -> [128, NDM*width]: out[r, width*ic + c] = aT[128ic+r, c]
        return np.ascontiguousarray(
            aT.reshape(NDM, 128, width).transpose(1, 0, 2).reshape(128, NDM * width)
        )

    wqr = pack_chunks(np.ascontiguousarray(Wq.T).astype(np.float16), D_HEAD)
    wkr = pack_chunks(np.ascontiguousarray(Wk.T).astype(np.float16), D_HEAD)
    # w2r[oc, r, 128*ic + o] = W2T[128ic+r, 128oc+o]
    w2T = np.ascontiguousarray(W2.T).astype(np.float16)
    w2r = np.ascontiguousarray(
        w2T.reshape(NDM, 128, NDM, 128).transpose(2, 1, 0, 3).reshape(NDM, 128, D_MODEL)
    )

    in_maps = []
    for c in range(NCORES):
        xqT = np.ascontiguousarray(x[c::NCORES].T).astype(np.float16)  # [D, QPC]
        xqr_c = pack_chunks(xqT, QPC)
        mask = np.zeros((128, 16), dtype=np.float32)
        j = np.arange(128)[:, None]
        t = np.arange(16)[None, :]
        mask[j > 8 * t + c] = MASK_NEG
        in_maps.append(
            {
                "xqr": xqr_c,
                "xtp": xtp,
                "xv": xv16,
                "wqr": wqr,
                "wkr": wkr,
                "w2r": w2r,
                "maskb": mask,
            }
        )
    return in_maps


def assemble_output(results):
    res = np.stack([np.asarray(results[c]["outT"]).astype(np.float32) for c in range(NCORES)])
    # [c, d, m] -> out[8m+c, d]
    return np.ascontiguousarray(res.transpose(2, 0, 1).reshape(N_CTX, D_MODEL))


_CACHED = {}


def kernel(x, Wk, Wq, W2, _trace=False):
    if "nc" not in _CACHED:
        _CACHED["nc"] = build_program()
    nc = _CACHED["nc"]
    in_maps = prepare_inputs(x, Wk, Wq, W2)
    res = run_bass_kernel_spmd(nc, in_maps, core_ids=list(range(NCORES)), trace=_trace)
    out = assemble_output(res.results)
    if _trace:
        return out, res
    return out



# revision 42
# speedup vs baseline: 1.2092x; 1.2092x over previous
"""Causal attention kernel for TRN2, 8 NeuronCores (SPMD).

Problem:  x[4096,2048] f32; q = x@Wq.T, k = x@Wk.T (d_head=128),
          scores = q@k.T causal-masked, attn = softmax(scores),
          out = (attn @ x) @ W2.T.

Sharding: sequence-parallel over queries with stride-8 interleave:
  core c owns queries {8m+c : m=0..511}.  For key tile kt (128 keys),
  every core has exactly 512-16*kt valid queries -- a contiguous tail
  slice of its query columns -- so the SPMD program is identical on all
  cores (no dynamic control flow, no collectives) and causal work is
  perfectly balanced.

Precision: fp16 inputs for the q/k projections and the score matmul
  (fp32 PSUM accumulation), unnormalized softmax (exp without
  max-subtraction: scores are bounded ~|s|<70 for unit-normal inputs,
  safely inside fp32 exp range), attention weights in bf16 (bf16 has
  fp32 exponent range, needed for exp(s) up to ~1e28), V and W2
  matmuls in bf16/fp16, normalization by the softmax row-sum applied
  at the attn_out eviction (keeps fp16 in range).

Scheduling notes (final, ~226us median on HW; 8-core uniform 221-232us):
  * All input streams are host-packed so every consumer group loads
    with one large contiguous DMA (HWDGE issue costs ~0.6us each; a
    naive per-tile version with 481 issues was issue-bound at 457us).
  * Byte schedule: small projection inputs -> xtp (packed x^T, pacing
    kT -> scores -> exp) -> xv (x natural, pacing the V matmul) -> w2r.
    xv is hard-ordered after the xT stream via an explicit dependency
    on the issuing sequencer: the machine is DMA-bound at ~270 GB/s/core
    through the first two streams, so any byte stealing from xtp slows
    the critical path 1:1 (measured both ways).
  * kT / attnT are split into per-keytile tiles so Tile's dependency
    tracking lets scores/exp/denominator/V pipeline into the DMA
    streams with no whole-tensor barriers.
  * The V matmul for output chunks 0-3 is fused into the score loop
    (PSUM: kT 2 + scores 1 + denom 1 + V 4 = 8 banks); chunks 4-15 run
    as kt-major groups on explicitly staged PSUM pools (stack
    allocator: group A reuses the fused pools' banks and overlaps the
    xv-paced tail of V[0:3]; B/C follow as banks release).  V
    accumulation groups execute start->stop in kt order, so xv must
    stream ascending.
  * Rejected variants (measured): AllGather-sharded kT (8-core 0.5MB
    AllGather costs ~100us here, replication is cheaper), interleaving
    xv into the xtp stream (fused phase is DMA-paced, no slack),
    descending xv (PSUM accumulation groups execute in emission order,
    the whole group waited for xv[0]).
"""

from contextlib import ExitStack

import numpy as np
import ml_dtypes

import concourse.bass as bass
import concourse.bacc as bacc
import concourse.mybir as mybir
import concourse.tile as tile
from concourse.bass_utils import run_bass_kernel_spmd
from concourse.tile_rust import add_dep_helper

N_CTX = 4096
D_MODEL = 2048
D_HEAD = 128
NCORES = 8
QPC = N_CTX // NCORES          # 512 queries per core
NKT = N_CTX // 128             # 32 key tiles
NDM = D_MODEL // 128           # 16 d_model chunks
KG = 256                       # kT projection key-group width
NKG = N_CTX // KG
MASK_NEG = -30000.0

F16 = mybir.dt.float16
BF16 = mybir.dt.bfloat16
F32 = mybir.dt.float32


def _widths():
    # valid query-column width per key tile (tail slice [512-w : 512] of qT)
    return [QPC - 16 * kt for kt in range(NKT)]


def build_program():
    nc = bacc.Bacc(trn_type="TRN2", target_bir_lowering=False, debug=False)

    # ---- DRAM parameters (identical shapes on all cores; data differs) ----
    # xqr[r, 512*ic + m] = x[8m+c, 128*ic + r]   (own-query columns, packed)
    xqr = nc.declare_dram_parameter("xqr", [128, NDM * QPC], F16, isOutput=False)
    # xtp[kg][r, KG*ic + n] = x[KG*kg + n, 128*ic + r]  (contiguous per-kg tiles)
    xtp = nc.declare_dram_parameter("xtp", [NKG, 128, NDM * KG], F16, isOutput=False)
    # xv = x (natural layout), bf16
    xv = nc.declare_dram_parameter("xv", [N_CTX, D_MODEL], BF16, isOutput=False)
    # wqr[r, 128*ic + h] = Wq[h, 128*ic + r]; same for wkr
    wqr = nc.declare_dram_parameter("wqr", [128, D_MODEL], F16, isOutput=False)
    wkr = nc.declare_dram_parameter("wkr", [128, D_MODEL], F16, isOutput=False)
    # w2r[oc][r, 128*ic + o] = W2[128*oc + o, 128*ic + r]
    w2r = nc.declare_dram_parameter("w2r", [NDM, 128, D_MODEL], F16, isOutput=False)
    maskb = nc.declare_dram_parameter("maskb", [128, 16], F32, isOutput=False)
    outT = nc.declare_dram_parameter("outT", [D_MODEL, QPC], F16, isOutput=True)

    W = _widths()

    with tile.TileContext(nc) as tc:
        with (
            tc.tile_pool(name="static", bufs=1) as st,
            tc.tile_pool(name="xvpool", bufs=NKT) as xvp,
            tc.tile_pool(name="ktpool", bufs=4) as ktp,
            tc.tile_pool(name="atpool", bufs=1) as atp,
        ):
            qT_sb = st.tile([128, QPC], F16, tag="qT")
            ones_sb = st.tile([128, 1], BF16, tag="ones")
            mask_sb = st.tile([128, 16], F32, tag="mask")
            recip_sb = st.tile([128, QPC], F32, tag="recip")
            nc.vector.memset(ones_sb[:], 1.0)

            # ---- byte plan: sync carries ONLY the critical in-order stream
            # [wk, xtp 0..15, xv 0..31, outs]; the q-proj inputs (mask, wq,
            # xq) ride the scalar queue in parallel during the cold start. ----
            es1 = ExitStack()  # SBUF transients: p1 + xts (freed before p34)
            p1 = es1.enter_context(tc.tile_pool(name="p1", bufs=1))
            wk_sb = p1.tile([128, D_MODEL], F16, tag="wk")
            nc.sync.dma_start(out=wk_sb[:], in_=wkr[:])
            wq_sb = p1.tile([128, D_MODEL], F16, tag="wq")
            nc.scalar.dma_start(out=wq_sb[:], in_=wqr[:])
            xq_sb = p1.tile([128, NDM * QPC], F16, tag="xq")
            for qq in range(2):
                nc.scalar.dma_start(
                    out=xq_sb[:, 8 * QPC * qq : 8 * QPC * (qq + 1)],
                    in_=xqr[:, 8 * QPC * qq : 8 * QPC * (qq + 1)],
                )
            nc.scalar.dma_start(out=mask_sb[:], in_=maskb[:])

            # xtp granularity is KG=256 keys (1MB per DMA): the stream is
            # consumer-gated, so per-DMA roundtrip (sem+issue+DGE+sem ~3us)
            # amortizes over 2x the bytes vs 128-key tiles -> ~350-400 GB/s.
            xts = es1.enter_context(tc.tile_pool(name="xts", bufs=4))
            xts_t = [None] * NKG
            for kg in range(NKG):
                t = xts.tile([128, NDM * KG], F16, tag="xts", name=f"xts{kg}")
                nc.sync.dma_start(out=t[:], in_=xtp[kg])
                xts_t[kg] = t

            # ---- xv loads strictly AFTER the xT stream (same sync queue, so
            # queue order == byte order; interleaving xv into the consumer-
            # gated xtp tail measured ~7us slower -- it stretches both
            # streams) ----
            xv_t = [None] * NKT
            last_xv_dma = None
            for kt in range(NKT):
                t = xvp.tile([128, D_MODEL], BF16, tag="xv", name=f"xv{kt}")
                last_xv_dma = nc.sync.dma_start(
                    out=t[:], in_=xv[128 * kt : 128 * (kt + 1), :]
                )
                xv_t[kt] = t

            # ---- k-proj for kg 0,1 hoisted BEFORE q-proj: the PE runs its
            # program in order, and xtp[0:2] bytes land before xq, so this
            # keeps the PE busy ~5us earlier than qproj-first. ----
            ktile_t = [None] * NKG
            with tc.tile_pool(name="psk", bufs=2, space="PSUM") as pskp:

                def emit_kproj(kg):
                    psk = pskp.tile([128, KG], F32, tag="psk", name=f"psk{kg}")
                    for ic in range(NDM):
                        nc.tensor.matmul(
                            psk[:],
                            wk_sb[:, 128 * ic : 128 * (ic + 1)],
                            xts_t[kg][:, KG * ic : KG * (ic + 1)],
                            start=(ic == 0), stop=(ic == NDM - 1),
                        )
                    ktile = ktp.tile([128, KG], F16, tag="kt", name=f"kt{kg}")
                    nc.vector.tensor_copy(ktile[:], psk[:])
                    ktile_t[kg] = ktile

                emit_kproj(0)
                emit_kproj(1)

                # ---- qT projection (2 halves, gated on the 2 xq chunks) ----
                with tc.tile_pool(name="psq", bufs=1, space="PSUM") as psqp:
                    psq = psqp.tile([128, QPC], F32, tag="psq")
                    for ic in range(NDM):
                        nc.tensor.matmul(
                            psq[:],
                            wq_sb[:, 128 * ic : 128 * (ic + 1)],
                            xq_sb[:, QPC * ic : QPC * (ic + 1)],
                            start=(ic == 0), stop=(ic == NDM - 1),
                        )
                    nc.vector.tensor_copy(qT_sb[:], psq[:])

                # ---- fused pipeline: kT proj / scores / exp / denom / V[0:4] ----
                es2 = ExitStack()  # PSUM: psv1 (freed mid-way through V chunk waves)
                psv1 = es2.enter_context(
                    tc.tile_pool(name="psv1", bufs=4, space="PSUM", side="right")
                )
                pso1 = [
                    psv1.tile([128, QPC], F32, tag="pso1", name=f"pso1_{j}")
                    for j in range(4)
                ]
                at_t = []
                scp = es1.enter_context(tc.tile_pool(name="scp", bufs=2))
                with (
                    tc.tile_pool(name="pss", bufs=1, space="PSUM") as pssp,
                    tc.tile_pool(name="psd", bufs=1, space="PSUM") as psdp,
                ):
                    psd = psdp.tile([1, QPC], F32, tag="psd")

                    def emit_scores(kg):
                        # scores -> SBUF copy -> mask -> exp for group kg.
                        # The PSUM score tile is copied to SBUF right away --
                        # the pss bank frees after one vector hop, and
                        # mask/exp run off the SBUF copy.
                        ktile = ktile_t[kg]
                        for sub in range(KG // 128):
                            kt = kg * (KG // 128) + sub
                            w = W[kt]
                            ps = pssp.tile([128, 512], F32, tag="pss", name=f"pss{kt}")
                            nc.tensor.matmul(
                                ps[:, :w],
                                ktile[:, 128 * sub : 128 * (sub + 1)],
                                qT_sb[:, QPC - w : QPC],
                                start=True, stop=True,
                            )
                            sc = scp.tile([128, 512], F16, tag="sc", name=f"sc{kt}")
                            nc.vector.tensor_copy(sc[:, :w], ps[:, :w])
                            nc.vector.tensor_add(sc[:, :16], sc[:, :16], mask_sb[:])
                            at = atp.tile([128, w], BF16, tag=f"at{kt}")
                            nc.scalar.activation(
                                at[:], sc[:, :w], mybir.ActivationFunctionType.Exp
                            )
                            at_t.append(at)

                    def emit_atv(kg):
                        # denominator + fused-V matmuls for group kg -- the
                        # at-tile consumers.  Emitted one group later than
                        # emit_scores so the PE (in-order) never waits on its
                        # own group's exp chain.
                        for sub in range(KG // 128):
                            kt = kg * (KG // 128) + sub
                            w = W[kt]
                            at = at_t[kt]
                            nc.tensor.matmul(
                                psd[0:1, QPC - w : QPC],
                                ones_sb[:],
                                at[:],
                                start=(kt == 0), stop=(kt == NKT - 1),
                            )
                            # V matmul for output chunks 0-3, fused
                            # (first key half only; the early eviction at xv[15]
                            # frees all 8 banks for the oc 8-15 full streams)
                            if kt < NKT // 2:
                                for j in range(4):
                                    nc.tensor.matmul(
                                        pso1[j][:, QPC - w : QPC],
                                        xv_t[kt][:, 128 * j : 128 * (j + 1)],
                                        at[:],
                                        start=(kt == 0), stop=(kt == NKT // 2 - 1),
                                    )

                    # software-pipelined emission: scores(kg) trail
                    # kproj(kg+2) and the at-consumers trail kproj(kg+3), so
                    # both the DMA->kproj and score->exp roundtrips hide
                    # behind kproj work (the PE executes its program in
                    # order).
                    for kg in range(2, NKG):
                        emit_kproj(kg)
                        emit_scores(kg - 2)
                        if kg >= 3:
                            emit_atv(kg - 3)
                    emit_scores(NKG - 2)
                    emit_atv(NKG - 3)
                    emit_scores(NKG - 1)
                    emit_atv(NKG - 2)
                    emit_atv(NKG - 1)

                    # copy the denominator row to SBUF immediately -- the psd
                    # bank is one of the four the bcl streams need, and the
                    # 1-partition reciprocal is slow (~3us); run it off SBUF
                    # (staged in a spare row of recip_sb) after the pool
                    # closes.
                    nc.vector.tensor_copy(recip_sb[0:1, :], psd[0:1, :])

            nc.vector.reciprocal(recip_sb[0:1, :], recip_sb[0:1, :])
            es1.close()  # free p1 + xts SBUF for the aoT / W2 pools
            nc.gpsimd.partition_broadcast(recip_sb[:], recip_sb[0:1, :])

            HK = NKT // 2
            with tc.tile_pool(name="p34", bufs=1) as p34:
                ao_t = {}

                # ---- oc 8-11 FULL-range V streams on the 4 left banks the
                # fused pools released, kt-synchronous with the xv stream
                # (xv tiles 0-15 are interleaved into the xtp tail, so these
                # start before the xT stream finishes) ----
                with tc.tile_pool(name="bcl", bufs=4, space="PSUM") as bclp:
                    bcl = {
                        oc: bclp.tile([128, QPC], F32, tag="bcl", name=f"bcl_{oc}")
                        for oc in range(8, 12)
                    }
                    for kt in range(HK):
                        w = W[kt]
                        for oc in range(8, 12):
                            nc.tensor.matmul(
                                bcl[oc][:, QPC - w : QPC],
                                xv_t[kt][:, 128 * oc : 128 * (oc + 1)],
                                at_t[kt][:],
                                start=(kt == 0), stop=False,
                            )
                    # early normalized evictions for oc 0-3 (first key half;
                    # exact for queries m<256 by causality), freeing the
                    # right-side fused-V banks
                    for j in range(4):
                        t = p34.tile([128, QPC], F16, tag=f"ao{j}")
                        nc.vector.tensor_mul(t[:], pso1[j][:], recip_sb[:])
                        ao_t[j] = t
                    es2.close()  # release the fused V banks

                    # ---- oc 4-7 first-half streams on the freed right banks,
                    # interleaved with bcl's second half: g2 reads resident
                    # xv[0:16] so the PE has work while the xv tail lands ----
                    with tc.tile_pool(name="g2", bufs=4, space="PSUM", side="right") as g2p:
                        g2 = {
                            oc: g2p.tile([128, QPC], F32, tag="g2", name=f"g2_{oc}")
                            for oc in range(4, 8)
                        }
                        for kt in range(HK, NKT):
                            w = W[kt]
                            for oc in range(8, 12):
                                nc.tensor.matmul(
                                    bcl[oc][:, QPC - w : QPC],
                                    xv_t[kt][:, 128 * oc : 128 * (oc + 1)],
                                    at_t[kt][:],
                                    start=False, stop=(kt == NKT - 1),
                                )
                            kg2 = kt - HK
                            wg = W[kg2]
                            for oc in range(4, 8):
                                nc.tensor.matmul(
                                    g2[oc][:, QPC - wg : QPC],
                                    xv_t[kg2][:, 128 * oc : 128 * (oc + 1)],
                                    at_t[kg2][:],
                                    start=(kg2 == 0), stop=(kg2 == HK - 1),
                                )
                        # bcl evictions FIRST (its stop lands with the last xv
                        # tile; the vector queue is in-order, so emitting them
                        # before g2/bcr evicts makes ao8-11 available ~25us
                        # earlier for W2's first groups)
                        for oc in range(8, 12):
                            t = p34.tile([128, QPC], F16, tag=f"ao{oc}")
                            nc.vector.tensor_mul(t[:], bcl[oc][:], recip_sb[:])
                            ao_t[oc] = t
                        for oc in range(4, 8):
                            t = p34.tile([128, QPC], F16, tag=f"ao{oc}")
                            nc.vector.tensor_mul(t[:], g2[oc][:], recip_sb[:])
                            ao_t[oc] = t

                    # ---- oc 12-15 full range on the right banks ----
                    with tc.tile_pool(name="bcr", bufs=4, space="PSUM", side="right") as bcrp:
                        bcr = {
                            oc: bcrp.tile([128, QPC], F32, tag="bcr", name=f"bcr_{oc}")
                            for oc in range(12, NDM)
                        }
                        for kt in range(NKT):
                            w = W[kt]
                            for oc in range(12, NDM):
                                nc.tensor.matmul(
                                    bcr[oc][:, QPC - w : QPC],
                                    xv_t[kt][:, 128 * oc : 128 * (oc + 1)],
                                    at_t[kt][:],
                                    start=(kt == 0), stop=(kt == NKT - 1),
                                )
                        for oc in range(12, NDM):
                            t = p34.tile([128, QPC], F16, tag=f"ao{oc}")
                            nc.vector.tensor_mul(t[:], bcr[oc][:], recip_sb[:])
                            ao_t[oc] = t

                # ---- oc 0-7, second key half (kt 16-31, queries [256:512]
                # only): dense from resident xv, merged into the early aos ----
                with (
                    tc.tile_pool(name="g4", bufs=4, space="PSUM") as g4p,
                    tc.tile_pool(name="tmr", bufs=4) as tmr,
                ):
                    for ocs in (range(0, 4), range(4, 8)):
                        g4 = {
                            oc: g4p.tile(
                                [128, QPC // 2], F32, tag="g4", name=f"g4_{oc}"
                            )
                            for oc in ocs
                        }
                        for kt in range(HK, NKT):
                            w = W[kt]
                            for oc in ocs:
                                nc.tensor.matmul(
                                    g4[oc][:, QPC // 2 - w : QPC // 2],
                                    xv_t[kt][:, 128 * oc : 128 * (oc + 1)],
                                    at_t[kt][:],
                                    start=(kt == HK), stop=(kt == NKT - 1),
                                )
                        for oc in ocs:
                            tm = tmr.tile([128, QPC // 2], F16, tag="tm")
                            nc.vector.tensor_mul(
                                tm[:], g4[oc][:], recip_sb[:, QPC // 2 :]
                            )
                            nc.vector.tensor_add(
                                ao_t[oc][:, QPC // 2 :],
                                ao_t[oc][:, QPC // 2 :],
                                tm[:],
                            )

                    # ---- W2: outT = W2T.T @ attn_outT.  ic order 8..15 first
                    # (those aos finish earliest), 0..7 after the merges ----
                    # All 16 w2 weight-load issues go on the scalar queue
                    # UPFRONT (ordered after the xv stream; tiles 7+ gate on
                    # buffer reuse) so no out-DMA issue can head-of-line
                    # block them; out DMAs ride the vector queue instead.
                    with (
                        tc.tile_pool(name="w2s", bufs=8) as w2s,
                        tc.tile_pool(name="outs", bufs=4) as outs,
                        tc.tile_pool(name="ps4", bufs=4, space="PSUM", side="right") as ps4,
                    ):
                        tw_t = []
                        for oc in range(NDM):
                            tw = w2s.tile([128, D_MODEL], F16, tag="w2",
                                          name=f"w2_{oc}")
                            dw = nc.scalar.dma_start(out=tw[:], in_=w2r[oc])
                            if oc == 0:
                                add_dep_helper(dw.ins, last_xv_dma.ins,
                                               reason="w2 bytes after xv stream")
                            tw_t.append(tw)
                        ic_order = list(range(8, NDM)) + list(range(0, 8))
                        for oc in range(NDM):
                            ps = ps4.tile([128, QPC], F32, tag="ps4")
                            for i, ic in enumerate(ic_order):
                                nc.tensor.matmul(
                                    ps[:],
                                    tw_t[oc][:, 128 * ic : 128 * (ic + 1)],
                                    ao_t[ic][:],
                                    start=(i == 0), stop=(i == NDM - 1),
                                )
                            t = outs.tile([128, QPC], F16, tag="out")
                            if oc == NDM - 1:
                                # split the final eviction so the copy/DMA
                                # chain pipelines instead of serializing the
                                # whole tail after the last matmul
                                for h in range(2):
                                    sl = slice(h * QPC // 2, (h + 1) * QPC // 2)
                                    nc.vector.tensor_copy(t[:, sl], ps[:, sl])
                                    nc.sync.dma_start(
                                        out=outT[128 * oc : 128 * (oc + 1), sl],
                                        in_=t[:, sl],
                                    )
                            else:
                                nc.vector.tensor_copy(t[:], ps[:])
                                nc.sync.dma_start(
                                    out=outT[128 * oc : 128 * (oc + 1), :], in_=t[:]
                                )

    nc.compile()
    return nc


def prepare_inputs(x, Wk, Wq, W2):
    """Host-side sharding/layout prep. Returns in_maps for the 8 cores."""
    x = np.asarray(x, dtype=np.float32)
    Wk = np.asarray(Wk, dtype=np.float32)
    Wq = np.asarray(Wq, dtype=np.float32)
    W2 = np.asarray(W2, dtype=np.float32)

    xT16 = np.ascontiguousarray(x.T).astype(np.float16)          # [D, N]
    # xtp[kg, r, KG*ic + n] = xT[128*ic + r, KG*kg + n]
    xtp = np.ascontiguousarray(
        xT16.reshape(NDM, 128, NKG, KG).transpose(2, 1, 0, 3).reshape(NKG, 128, NDM * KG)
    )
    xv16 = x.astype(ml_dtypes.bfloat16)                          # [N, D]

    def pack_chunks(aT, width):
        # aT [D_MODEL, width] -> [128, NDM*width]: out[r, width*ic + c] = aT[128ic+r, c]
        return np.ascontiguousarray(
            aT.reshape(NDM, 128, width).transpose(1, 0, 2).reshape(128, NDM * width)
        )

    wqr = pack_chunks(np.ascontiguousarray(Wq.T).astype(np.float16), D_HEAD)
    wkr = pack_chunks(np.ascontiguousarray(Wk.T).astype(np.float16), D_HEAD)
    # w2r[oc, r, 128*ic + o] = W2T[128ic+r, 128oc+o]
    w2T = np.ascontiguousarray(W2.T).astype(np.float16)
    w2r = np.ascontiguousarray(
        w2T.reshape(NDM, 128, NDM, 128).transpose(2, 1, 0, 3).reshape(NDM, 128, D_MODEL)
    )

    in_maps = []
    for c in range(NCORES):
        xqT = np.ascontiguousarray(x[c::NCORES].T).astype(np.float16)  # [D, QPC]
        xqr_c = pack_chunks(xqT, QPC)
        mask = np.zeros((128, 16), dtype=np.float32)
        j = np.arange(128)[:, None]
        t = np.arange(16)[None, :]
        mask[j > 8 * t + c] = MASK_NEG
        in_maps.append(
            {
                "xqr": xqr_c,
                "xtp": xtp,
                "xv": xv16,
                "wqr": wqr,
                "wkr": wkr,
                "w2r": w2r,
                "maskb": mask,
            }
        )
    return in_maps


def assemble_output(results):
    res = np.stack([np.asarray(results[c]["outT"]).astype(np.float32) for c in range(NCORES)])
    # [c, d, m] -> out[8m+c, d]
    return np.ascontiguousarray(res.transpose(2, 0, 1).reshape(N_CTX, D_MODEL))


_CACHED = {}


def kernel(x, Wk, Wq, W2, _trace=False):
    if "nc" not in _CACHED:
        _CACHED["nc"] = build_program()
    nc = _CACHED["nc"]
    in_maps = prepare_inputs(x, Wk, Wq, W2)
    res = run_bass_kernel_spmd(nc, in_maps, core_ids=list(range(NCORES)), trace=_trace)
    out = assemble_output(res.results)
    if _trace:
        return out, res
    return out

